# revision 1
# baseline (speedup 1.0000x reference)
"""Trainium2 Bass kernel for bidirectional Chamfer distance (B=8, N=M=8192).

Sharding: data-parallel over batch -- one NeuronCore per batch element; the
host combines the 8 cores' per-point minima (all-reduce of the scalar means
is O(N) host work).

Per core, both directions of the chamfer min run as two matmul orientations
(weights=targets / weights=preds) of an augmented K=24 matmul that emits
finished 128x512 squared-distance chunks straight into PSUM:

    dist(n, m) = p_sq[n] + t_sq[m] - 2 <p_n, t_m>

Numerics: every augmented row is split into three bf16 parts (hi/mid/lo), so
each fp32 input is represented exactly to ~2^-25 and all bf16 products are
exact in the PE's fp32 accumulate -> fp32-level accuracy at bf16 streaming
speed (1 cycle/row).  K=24 <= 32 lets four matmuls run concurrently in the
PE's four 32-row groups (tile_position=(32i,0)), one PSUM bank each (~4x PE
throughput).

Reduction (the bottleneck): the Scalar engine drains ALL of PSUM, casting
each 4-bank [128,2048] fp32 tile to f16 in SBUF (measured ~0.5 cycle/elem
@1.2GHz); the Vector engine never touches PSUM and runs exclusively in its
2x f16 mode: one tensor_tensor_reduce per pair of cast blocks computes
min(c0,c1) AND folds the [128,1] running min via its accum_out, chained
through the `scalar` initial-value AP -- two TTRs per 128-point tile, no
reduction tail.  Host applies max(.,0) + means.
"""

import ml_dtypes
import numpy as np

import concourse.bass as bass
import concourse.mybir as mybir
import concourse.tile as tile
from concourse import bacc
from concourse.bass_utils import run_bass_kernel_spmd

try:  # persistent jit/NEFF cache: makes repeat invocations fast
    import jax

    jax.config.update("jax_compilation_cache_dir", "/tmp/.jax_bass_cache")
    jax.config.update("jax_persistent_cache_min_compile_time_secs", 1.0)
except Exception:
    pass

F32 = mybir.dt.float32
F16 = mybir.dt.float16
BF16 = mybir.dt.bfloat16
MIN = mybir.AluOpType.min
BIG = 3.0e38

B, N, M = 8, 8192, 8192
KROWS = 24
CHUNK = 512


def _build_nc(N=8192, M=8192, chunk=512, fd=2048, repeat=1, cast_bufs=4,
              merge_bufs=4, psum_bufs=2, mode="full", stage_dt="f16",
              gp=0, hybrid=True):
    """Inputs (per core), all [128, n] bf16 with the 24 aug rows replicated at
    partition offsets 0/32/64/96:
      wa: aug-weights(target) [128, M]   (orientation A: out[m_part, n_free])
      sa: aug-stream(pred)    [128, N]
      wb: aug-weights(pred)   [128, N]   (orientation B: out[n_part, m_free])
      sb: aug-stream(target)  [128, M]
    Output: mins [128, M/128 + N/128] fp32.
    """
    del hybrid  # kept for call-site compat; this kernel has a single path
    assert N % (2 * fd) == 0 and M % (2 * fd) == 0 and fd % chunk == 0
    nta = M // 128
    ntb = N // 128
    cpg = fd // chunk  # chunks per psum tile
    SDT = F16 if stage_dt == "f16" else F32

    nc = bacc.Bacc("TRN2", target_bir_lowering=False, debug=False)
    wa = nc.dram_tensor("wa", [128, M], BF16, kind="ExternalInput").ap()
    sa = nc.dram_tensor("sa", [128, N], BF16, kind="ExternalInput").ap()
    wb = nc.dram_tensor("wb", [128, N], BF16, kind="ExternalInput").ap()
    sb = nc.dram_tensor("sb", [128, M], BF16, kind="ExternalInput").ap()
    out = nc.dram_tensor("mins", [128, nta + ntb], F32, kind="ExternalOutput").ap()

    with tile.TileContext(nc) as tc:
        with (
            tc.tile_pool(name="const", bufs=1) as const_pool,
            tc.tile_pool(name="psum", bufs=psum_bufs, space="PSUM") as psum_pool,
            tc.tile_pool(name="cast", bufs=cast_bufs) as cast_pool,
            tc.tile_pool(name="merge", bufs=merge_bufs) as merge_pool,
            tc.tile_pool(name="tail", bufs=4) as tail_pool,
            tc.tile_pool(name="res", bufs=1) as res_pool,
        ):
            sb_t = {}
            for name, dram in (("wa", wa), ("sa", sa), ("wb", wb), ("sb", sb)):
                t = const_pool.tile([128, dram.shape[1]], BF16, tag=name)
                nc.sync.dma_start(t[:], dram[:])
                sb_t[name] = t

            res = res_pool.tile([128, nta + ntb], F32)

            # 'noact' diagnostic: merges fold constant SBUF tiles (wrong
            # result; isolates pure DVE merge+tail throughput, no ACT/PSUM)
            csts = None
            if mode == "noact":
                csts = []
                for i in range(4):
                    cst = const_pool.tile([128, fd], SDT, tag=f"cst{i}", name=f"cst{i}")
                    nc.vector.memset(cst[:], 1.0)
                    csts.append(cst)

            def fill(ps, w, s, t, base_chunk, rp0=0):
                # one PSUM tile <- cpg concurrent row-group matmuls
                for i in range(cpg):
                    rp = 32 * (rp0 + i)
                    c = base_chunk + i
                    nc.tensor.matmul(
                        ps[:, i * chunk : (i + 1) * chunk],
                        lhsT=w[rp : rp + KROWS, t * 128 : (t + 1) * 128],
                        rhs=s[rp : rp + KROWS, c * chunk : (c + 1) * chunk],
                        start=True,
                        stop=True,
                        tile_position=(rp, 0),
                    )

            for _rep in range(repeat):
              for wname, sname, ntiles, col0 in (
                ("wa", "sa", nta, 0),
                ("wb", "sb", ntb, nta),
              ):
                w = sb_t[wname]
                s = sb_t[sname]
                n_stream = s.shape[1]
                ngroups = n_stream // fd  # 4-bank groups per tile (4)
                assert ngroups % 2 == 0
                assert ngroups == 4, "merge tree below assumes 4 cast blocks"
                for t in range(ntiles):
                    rescol = res[:, col0 + t : col0 + t + 1]
                    if mode == "noact":
                        blocks = csts
                    else:
                        blocks = []
                        for g in range(ngroups):
                            ps = psum_pool.tile([128, fd], F32, tag="ps")
                            fill(ps, w, s, t, g * cpg, rp0=(g * cpg) % 4)
                            c = cast_pool.tile([128, fd], SDT, tag="c")
                            nc.scalar.copy(c[:], ps[:])
                            blocks.append(c)
                        if mode == "nodve":
                            nc.scalar.copy(rescol, blocks[-1][:, :1])
                            continue
                    m0 = merge_pool.tile([128, fd], SDT, tag="m")
                    nc.vector.tensor_tensor(m0[:], blocks[0][:], blocks[1][:], op=MIN)
                    m1 = merge_pool.tile([128, fd], SDT, tag="m")
                    nc.vector.tensor_tensor(m1[:], blocks[2][:], blocks[3][:], op=MIN)
                    acc = merge_pool.tile([128, fd], SDT, tag="m")
                    nc.vector.tensor_tensor(acc[:], m0[:], m1[:], op=MIN)
                    # tail: fd -> 1; first `gp` halvings on GpSimd, rest DVE
                    width = fd
                    cur = acc
                    for lvl in range(2):
                        half = width // 2
                        h = tail_pool.tile([128, half], SDT, tag="t")
                        eng = nc.gpsimd if lvl < gp else nc.vector
                        eng.tensor_tensor(
                            h[:], cur[:, :half], cur[:, half:], op=MIN
                        )
                        cur = h
                        width = half
                    nc.vector.tensor_reduce(
                        rescol, cur[:], axis=mybir.AxisListType.X, op=MIN
                    )

            nc.sync.dma_start(out[:], res[:])

    nc.compile()
    return nc


def _split3(x):
    """fp32 -> (hi, mid, lo) bf16 parts with hi+mid+lo == x to ~2^-25 rel."""
    x = np.asarray(x, np.float32)
    h = x.astype(ml_dtypes.bfloat16)
    r = x - h.astype(np.float32)
    m = r.astype(ml_dtypes.bfloat16)
    l = (r - m.astype(np.float32)).astype(ml_dtypes.bfloat16)
    return h, m, l


def _aug24(w_pts, s_pts, w_sq, s_sq):
    """K=24 bf16 weight/stream matrices for one orientation (w side gets -2)."""
    Mw = w_pts.shape[0]
    Ns = s_pts.shape[0]
    W = np.zeros((KROWS, Mw), ml_dtypes.bfloat16)
    S = np.zeros((KROWS, Ns), ml_dtypes.bfloat16)
    one_w = np.ones(Mw, ml_dtypes.bfloat16)
    one_s = np.ones(Ns, ml_dtypes.bfloat16)

    W[0], W[1], W[2] = _split3(w_sq)
    S[0], S[1], S[2] = one_s, one_s, one_s
    W[3], W[4], W[5] = one_w, one_w, one_w
    S[3], S[4], S[5] = _split3(s_sq)

    for c in range(3):
        vh, vm, vl = _split3((-2.0 * w_pts[:, c]).astype(np.float32))
        ph, pm, pl = _split3(s_pts[:, c])
        r = 6 + 6 * c
        W[r + 0], S[r + 0] = vh, ph
        W[r + 1], S[r + 1] = vh, pm
        W[r + 2], S[r + 2] = vm, ph
        W[r + 3], S[r + 3] = vh, pl
        W[r + 4], S[r + 4] = vl, ph
        W[r + 5], S[r + 5] = vm, pm
    return W, S


def _replicate4(A):
    """[24, n] -> [128, n] with copies at partition offsets 0/32/64/96."""
    out = np.zeros((128, A.shape[1]), ml_dtypes.bfloat16)
    for i in range(4):
        out[32 * i : 32 * i + KROWS] = A
    return out


def _augment(pred_b, target_b):
    """Host-side O(N) prep for one batch -> four [128, n] bf16 arrays."""
    p = np.asarray(pred_b, np.float32)
    t = np.asarray(target_b, np.float32)
    p_sq = (p.astype(np.float64) ** 2).sum(axis=1).astype(np.float32)
    t_sq = (t.astype(np.float64) ** 2).sum(axis=1).astype(np.float32)
    WA, SA = _aug24(t, p, t_sq, p_sq)  # orientation A: weights = targets
    WB, SB = _aug24(p, t, p_sq, t_sq)  # orientation B: weights = preds
    return {
        "wa": _replicate4(WA),
        "sa": _replicate4(SA),
        "wb": _replicate4(WB),
        "sb": _replicate4(SB),
    }


_NC_CACHE = {}


def _get_nc():
    if "nc" not in _NC_CACHE:
        _NC_CACHE["nc"] = _build_nc()
    return _NC_CACHE["nc"]


def kernel(pred: np.ndarray, target: np.ndarray) -> np.ndarray:
    pred = np.asarray(pred, np.float32)
    target = np.asarray(target, np.float32)
    assert pred.shape == (B, N, 3) and target.shape == (B, M, 3), (
        pred.shape,
        target.shape,
    )

    nc = _get_nc()
    in_maps = [_augment(pred[b], target[b]) for b in range(B)]
    results = run_bass_kernel_spmd(nc, in_maps, list(range(B))).results

    nta = M // 128
    t2p = []  # per-target minima (min over preds)
    p2t = []  # per-pred minima (min over targets)
    for b in range(B):
        mins = results[b]["mins"]
        t2p.append(np.maximum(mins[:, :nta], 0.0).reshape(-1))
        p2t.append(np.maximum(mins[:, nta:], 0.0).reshape(-1))
    cd = np.mean(np.concatenate(p2t), dtype=np.float64) + np.mean(
        np.concatenate(t2p), dtype=np.float64
    )
    return np.array(cd, dtype=np.float32)

